# revision 3
# baseline (speedup 1.0000x reference)
"""Trainium2 Bass kernel for nn_BondPredictor (GNN message passing).

Sharding: edges are assigned to the core owning their dst node (8 contiguous
node ranges). Message aggregation (segment_sum by dst) is then fully local to
a core and done on-chip via selection-matrix matmuls into per-window PSUM
accumulators. Node states are updated on the owning core and replicated with
one AllGather (bf16 table) per layer; gathers of endpoint features use
transposing dma_gather straight into feature-major SBUF tiles. The half-edge
symmetrization for the decoder is a single AllToAll of final edge states.
"""
import sys, os
sys.path.insert(0, '/opt/trn_rl_repo')
import numpy as np
import ml_dtypes

bf16 = ml_dtypes.bfloat16

# problem constants
N, EH, G = 20000, 100000, 1000
E = 2 * EH
NT, ND, ED, L, NE = 16, 256, 128, 2, 5
TIME_DIM = DIST_DIM = 16
NUM_T = 1000

NC = 8
RANGE = 2560                  # nodes per core (20 windows of 128)
NWIN = RANGE // 128           # 20
NPAD = NC * RANGE             # 20480
PAD_NODE = NPAD               # row of zeros
TROWS = NPAD + 128            # node table rows (pad rows zero)
CHUNK = 2048                  # edge chunk (16 subtiles of 128, 4 n-tiles of 512)
DEC_CHUNK = 512               # decoder pair padding granularity

T_COEFF = -0.5 / (NUM_T / (TIME_DIM - 1)) ** 2
D_COEFF = -0.5 / (10.0 / (DIST_DIM - 1)) ** 2
T_OFFS = np.linspace(0.0, float(NUM_T), TIME_DIM).astype(np.float32)
D_OFFS = np.linspace(0.0, 10.0, DIST_DIM).astype(np.float32)


def _wrap16(v, pad_to=None):
    """Host int index list -> [128, n/16] int16 SBUF layout for dma_gather."""
    v = np.asarray(v, np.int64)
    if pad_to is not None and len(v) < pad_to:
        v = np.concatenate([v, np.zeros(pad_to - len(v), np.int64)])
    n = len(v)
    assert n % 16 == 0
    a = np.zeros((16, n // 16), np.int16)
    for p in range(16):
        a[p, :] = v[p::16]
    return np.tile(a, (8, 1))


def _prep(h_node, pos_node, batch_node, edge_index, batch_edge, t):
    """All host-side layout: sharding, sorting, padding, index arrays."""
    src = np.asarray(edge_index[0], np.int64)
    dst = np.asarray(edge_index[1], np.int64)
    t = np.asarray(t, np.float32)
    te_all = t[np.asarray(batch_edge, np.int64)]          # [E]
    tn_all = t[np.asarray(batch_node, np.int64)]          # [N]
    h_node = np.asarray(h_node, np.float32)
    pos_node = np.asarray(pos_node, np.float32)

    core_of = dst // RANGE                                 # [E] 0..7
    win_of = (dst - core_of * RANGE) // 128                # [E] 0..19

    # global per-window budgets
    counts = np.zeros((NC, NWIN), np.int64)
    for k in range(NC):
        m = core_of == k
        counts[k] = np.bincount(win_of[m], minlength=NWIN)
    B = counts.max(axis=0)                                 # [NWIN]
    total = int(B.sum())
    nchunk = -(-total // CHUNK)
    E_pad = nchunk * CHUNK
    B = B.copy()
    B[-1] += E_pad - total                                 # absorb into last window region
    off = np.concatenate([[0], np.cumsum(B)])              # window region offsets

    # subtile -> window span (uniform across cores)
    nsub = E_pad // 128
    w_of_slot = np.searchsorted(off[1:], np.arange(E_pad), side='right')
    w_of_slot = np.minimum(w_of_slot, NWIN - 1)
    sub_wlo = w_of_slot[np.arange(nsub) * 128]
    sub_whi = w_of_slot[np.arange(nsub) * 128 + 127]
    # window close subtile
    win_close = np.zeros(NWIN, np.int64)
    for w in range(NWIN):
        win_close[w] = np.nonzero((sub_wlo <= w) & (sub_whi >= w))[0][-1]

    cores = []
    slot_of_edge = np.full(E, -1, np.int64)
    for k in range(NC):
        s_src = np.full(E_pad, PAD_NODE, np.int64)
        s_dst = np.full(E_pad, PAD_NODE, np.int64)
        s_dsel = np.full(E_pad, -1.0, np.float32)
        s_orig = np.full(E_pad, -1, np.int64)
        for w in range(NWIN):
            eids = np.nonzero((core_of == k) & (win_of == w))[0]
            eids = eids[np.argsort(dst[eids], kind='stable')]
            sl = off[w] + np.arange(len(eids))
            s_src[sl] = src[eids]
            s_dst[sl] = dst[eids]
            s_dsel[sl] = dst[eids] - k * RANGE
            s_orig[sl] = eids
            slot_of_edge[eids] = sl
        real = s_orig >= 0
        te = np.where(real, te_all[np.clip(s_orig, 0, E - 1)], 0.0).astype(np.float32)
        ps = np.zeros((E_pad, 4), np.float32)
        pd = np.zeros((E_pad, 4), np.float32)
        ps[real, :3] = pos_node[s_src[real]]
        pd[real, :3] = pos_node[s_dst[real]]
        oh2 = np.zeros((32, E_pad), np.float32)
        oh2[:16, real] = h_node[s_src[real]].T
        oh2[16:, real] = h_node[s_dst[real]].T
        cores.append(dict(
            s_src=s_src, s_dst=s_dst, s_dsel=s_dsel, s_orig=s_orig,
            te=te, ps=ps, pd=pd, oh2=oh2,
        ))

    # node slices
    for k in range(NC):
        lo, hi = k * RANGE, min((k + 1) * RANGE, N)
        hnT = np.zeros((16, RANGE), np.float32)
        tn = np.zeros(RANGE, np.float32)
        if hi > lo:
            hnT[:, :hi - lo] = h_node[lo:hi].T
            tn[:hi - lo] = tn_all[lo:hi]
        cores[k]['hnT'] = hnT
        cores[k]['tn'] = tn

    # ---- decoder / pairing
    home = core_of[:EH]                                    # pair p home = owner(dst[e1])
    j2 = core_of[EH:]                                      # core holding e2 = p+EH
    P_k = np.bincount(home, minlength=NC)
    P_pad = -(-int(P_k.max()) // DEC_CHUNK) * DEC_CHUNK
    pair_lists = [np.nonzero(home == k)[0] for k in range(NC)]

    # sender buckets: for core j (holds e2 of pairs with j2==j), to core k (home)
    bucket = [[None] * NC for _ in range(NC)]              # bucket[j][k] = pair ids
    for k in range(NC):
        pk = pair_lists[k]
        for j in range(NC):
            bucket[j][k] = pk[j2[pk] == j]                 # in receiver pair order
    S_max = max(len(bucket[j][k]) for j in range(NC) for k in range(NC))
    S_pad = -(-S_max // 128) * 128

    for k in range(NC):
        pk = pair_lists[k]
        npk = len(pk)
        # e1 slot (local) per pair
        e1s = np.zeros(P_pad, np.int64)
        e1s[:npk] = slot_of_edge[pk]
        # receiver-side flat recv index per pair: j2*S_pad + pos within bucket
        pos = np.zeros(P_pad, np.int64)
        for j in range(NC):
            b = bucket[j][k]
            ppos = np.searchsorted(pk, b)                  # positions of b in pk
            pos[ppos] = j * S_pad + np.arange(len(b))
        ds = np.full(P_pad, PAD_NODE, np.int64)
        dd = np.full(P_pad, PAD_NODE, np.int64)
        ds[:npk] = src[pk]
        dd[:npk] = dst[pk]
        # sender-side gather order: concat over dest k of local e2 slots
        sg = np.zeros(NC * S_pad, np.int64)
        for dk in range(NC):
            b = bucket[k][dk]
            sg[dk * S_pad:dk * S_pad + len(b)] = slot_of_edge[b + EH]
        cores[k].update(dict(
            pairs=pair_lists[k], e1s=e1s, rg=pos, dsrc=ds, ddst=dd, sg=sg,
        ))

    meta = dict(E_pad=E_pad, nchunk=nchunk, nsub=nsub, B=B, off=off,
                sub_wlo=sub_wlo, sub_whi=sub_whi, win_close=win_close,
                P_pad=P_pad, S_pad=S_pad)
    return cores, meta


def _gs(x, offs, coeff):
    return np.exp(coeff * (x[:, None] - offs[None, :]) ** 2)


def emulate(inputs):
    """Pure-numpy f32 emulation of the sharded computation (dev validation)."""
    cores, meta = _prep(inputs['h_node'], inputs['pos_node'], inputs['batch_node'],
                        inputs['edge_index'], inputs['batch_edge'], inputs['t'])
    E_pad, P_pad, S_pad = meta['E_pad'], meta['P_pad'], meta['S_pad']
    W = {k: np.asarray(v, np.float32) for k, v in inputs.items()}

    # embed: node table
    table = np.zeros((TROWS, 256), np.float32)
    hn = []
    for k in range(NC):
        c = cores[k]
        emb = c['hnT'].T @ W['W_node_emb']                 # [RANGE, 240]
        tem = _gs(c['tn'], T_OFFS, T_COEFF)
        hv = np.concatenate([emb, tem], axis=1)            # [RANGE, 256]
        hn.append(hv)
        table[k * RANGE:(k + 1) * RANGE] = hv
    # edge states
    he, c17 = [], []
    for k in range(NC):
        c = cores[k]
        h0 = c['oh2'].T @ W['W_edge_emb']                  # [E_pad, 112]
        tem = _gs(c['te'], T_OFFS, T_COEFF)
        he.append(np.concatenate([h0, tem], axis=1))       # [E_pad, 128]
        d = np.linalg.norm((c['pd'] - c['ps'])[:, :3], axis=1)
        de = _gs(d, D_OFFS, D_COEFF)
        c17.append(np.concatenate([de, (c['te'] / NUM_T)[:, None]], axis=1))

    for l in range(L):
        new_slices = []
        for k in range(NC):
            c = cores[k]
            hs = table[c['s_src']]
            hd = table[c['s_dst']]
            e_in = np.concatenate([he[k], hs, hd, c17[k]], axis=1)
            he[k] = he[k] + np.maximum(e_in @ W['We1'][l] + W['be1'][l], 0) @ W['We2'][l] + W['be2'][l]
            msg = np.maximum(np.concatenate([hs, he[k]], axis=1) @ W['Wm'][l] + W['bm'][l], 0)
            agg = np.zeros((RANGE, 256), np.float32)
            real = c['s_dsel'] >= 0
            np.add.at(agg, c['s_dsel'][real].astype(np.int64), msg[real])
            n_in = np.concatenate([hn[k], agg, (c['tn'] / NUM_T)[:, None]], axis=1)
            upd = np.maximum(n_in @ W['Wn1'][l] + W['bn1'][l], 0) @ W['Wn2'][l] + W['bn2'][l]
            new_slices.append(hn[k] + upd)
        for k in range(NC):
            hn[k] = new_slices[k]
            table[k * RANGE:(k + 1) * RANGE] = hn[k]

    # A2A exchange of final he rows
    send = [np.zeros((NC * S_pad, 128), np.float32) for _ in range(NC)]
    for k in range(NC):
        send[k][:] = he[k][cores[k]['sg']]
    recv = [np.zeros((NC * S_pad, 128), np.float32) for _ in range(NC)]
    for k in range(NC):
        for j in range(NC):
            recv[k][j * S_pad:(j + 1) * S_pad] = send[j][k * S_pad:(k + 1) * S_pad]

    out = np.zeros((EH, NE), np.float32)
    for k in range(NC):
        c = cores[k]
        he1 = he[k][c['e1s']]
        he2 = recv[k][c['rg']]
        hes = he1 + he2
        hns = table[c['dsrc']] + table[c['ddst']]
        hx = np.concatenate([hes, hns], axis=1)
        h = np.maximum(hx @ W['Wd1'] + W['bd1'], 0)
        h = np.maximum(h @ W['Wd2'] + W['bd2'], 0)
        o = h @ W['Wd3'] + W['bd3']
        npk = len(c['pairs'])
        out[c['pairs']] = o[:npk]
    return out


def kernel(**inputs):
    from kernel_device import run_device
    out, _ = run_device(inputs)
    return out


if __name__ == '__main__':
    pass
